# revision 1
# baseline (speedup 1.0000x reference)
"""Trainium2 Bass kernel: inclusive cumsum along L for X (4, 8192, 32, 32) f32.

Strategy (8 NeuronCores, SPMD):
  - View X as (B=4, L=8192, C=1024) with C = D*N flattened. The 4096 scan
    rows (b, c) are independent; shard them 8 ways: core i gets b = i//2 and
    the c-half h = i%2, i.e. a (8192, 512) slab whose DRAM rows are 2 KiB
    contiguous.
  - Per core: stream 512-long L superblocks (1 MiB batched DMAs). Each
    (128 l, 128 c) sub-tile is PE-transposed into PSUM banks laid out as
    (128 c, 512 l). The VectorE tensor_tensor_scan runs the cumsum along the
    free (l) dim, chaining superblocks via the `initial` operand (carry =
    last column of the previous scan output). Scan outputs are PE-transposed
    back to natural (l, c) layout in PSUM, copied to SBUF by ScalarE, and
    DMA'd out as 1 MiB transfers.
  - Engine budget per core (measured): DMA ~94-105 us saturated at the
    ~358 GB/s HBM-per-core limit (the bottleneck), DVE scans ~81 us, PE
    transposes ~70 us, ACT copies ~36 us, plus ~8 us NEFF preamble and
    ~9 us Tile exit barrier. In-DMAs rotate across the Sync/GPSIMD/Scalar
    issue paths and out-DMAs alternate GPSIMD/Sync for DMA-queue diversity.
    Measured ~112 us mean / ~115 us slowest-core on 8 NC-v3 cores.
"""

import numpy as np
from contextlib import ExitStack

import concourse.bass as bass
import concourse.tile as tile
from concourse import bacc, masks, mybir
from concourse.bass_utils import run_bass_kernel_spmd

N_CORES = 8
B, L, D, N = 4, 8192, 32, 32
C_FULL = D * N          # 1024 columns per batch entry
C = C_FULL // 2         # 512 columns per core
P = 128                 # partitions
SUPER = 512             # L elems per superblock
N_SUPER = L // SUPER    # 16
BLKS = SUPER // P       # 4 L-blocks per superblock
CGRP = C // P           # 4 column groups per core

_CACHE = {}


def _build_program():
    f32 = mybir.dt.float32
    nc = bacc.Bacc(
        trn_type="TRN2", debug=False, num_devices=N_CORES, num_swdge_queues=2
    )
    x = nc.dram_tensor("x", [L, C], f32, kind="ExternalInput").ap()
    y = nc.dram_tensor("y", [L, C], f32, kind="ExternalOutput").ap()

    with tile.TileContext(nc) as tc, ExitStack() as ctx:
        const_pool = ctx.enter_context(tc.tile_pool(name="const", bufs=1))
        xin_pool = ctx.enter_context(tc.tile_pool(name="xin", bufs=6))
        scano_pool = ctx.enter_context(tc.tile_pool(name="scano", bufs=2))
        yout_pool = ctx.enter_context(tc.tile_pool(name="yout", bufs=6))
        inps_pool = ctx.enter_context(tc.tile_pool(name="inps", bufs=4, space="PSUM"))
        outps_pool = ctx.enter_context(tc.tile_pool(name="outps", bufs=2, space="PSUM"))

        ident = const_pool.tile([P, P], f32, name="ident")
        masks.make_identity(nc, ident[:])
        zeros = const_pool.tile([P, SUPER], f32, name="zeros")
        nc.gpsimd.memset(zeros[:], 0.0)

        prev = [None] * CGRP
        for t in range(N_SUPER):
            # ---- load the whole superblock with one 1 MiB DMA ----
            # DRAM rows l = t*512 + ks*128 + p; element order [p][ks][c] on
            # both sides so the 3D APs pair up.
            xt = xin_pool.tile([P, BLKS * C], f32, name="xt", tag="xt", bufs=6)
            src = x[t * SUPER : (t + 1) * SUPER, :].rearrange(
                "(ks p) c -> p ks c", p=P
            )
            dst = xt[:].rearrange("p (ks c) -> p ks c", ks=BLKS)
            in_eng = (nc.sync, nc.gpsimd, nc.scalar)[t % 3]
            in_eng.dma_start(out=dst, in_=src)

            # ---- transpose into (c, l) PSUM banks; scan along l ----
            souts = []
            for j in range(CGRP):
                ib = inps_pool.tile([P, SUPER], f32, name="ib", tag="ib", bufs=4)
                for ks in range(BLKS):
                    nc.tensor.transpose(
                        ib[:, ks * P : (ks + 1) * P],
                        xt[:, ks * C + j * P : ks * C + (j + 1) * P],
                        ident[:],
                    )
                so = scano_pool.tile(
                    [P, SUPER], f32, name=f"so{j}", tag=f"so{j}", bufs=2
                )
                init = 0.0 if t == 0 else prev[j][:, SUPER - 1 : SUPER]
                nc.vector.tensor_tensor_scan(
                    so[:], ib[:], zeros[:], init,
                    mybir.AluOpType.add, mybir.AluOpType.add,
                )
                souts.append(so)
            prev = souts

            # ---- transpose back to (l, c); ScalarE copies PSUM->SBUF ----
            yt = yout_pool.tile([P, BLKS * C], f32, name="yt", tag="yt", bufs=6)
            for half in range(2):
                ob = outps_pool.tile([P, 2 * C], f32, name="ob", tag="ob", bufs=2)
                for i2 in range(2):
                    i = half * 2 + i2
                    for j in range(CGRP):
                        nc.tensor.transpose(
                            ob[:, i2 * C + j * P : i2 * C + (j + 1) * P],
                            souts[j][:, i * P : (i + 1) * P],
                            ident[:],
                        )
                nc.scalar.copy(yt[:, half * 2 * C : (half + 1) * 2 * C], ob[:])

            ydst = y[t * SUPER : (t + 1) * SUPER, :].rearrange(
                "(ks p) c -> p ks c", p=P
            )
            ysrc = yt[:].rearrange("p (ks c) -> p ks c", ks=BLKS)
            out_eng = nc.gpsimd if t % 2 == 0 else nc.sync
            out_eng.dma_start(out=ydst, in_=ysrc)

    nc.compile()
    return nc


def _get_program():
    if "nc" not in _CACHE:
        _CACHE["nc"] = _build_program()
    return _CACHE["nc"]


def _shard(X):
    """(4, 8192, 32, 32) -> 8 contiguous (8192, 512) slabs."""
    Xv = X.reshape(B, L, C_FULL)
    shards = []
    for i in range(N_CORES):
        b, h = i // 2, i % 2
        shards.append(np.ascontiguousarray(Xv[b, :, h * C : (h + 1) * C]))
    return shards


def _unshard(parts):
    out = np.empty((B, L, C_FULL), dtype=np.float32)
    for i in range(N_CORES):
        b, h = i // 2, i % 2
        out[b, :, h * C : (h + 1) * C] = parts[i]
    return out.reshape(B, L, D, N)


def kernel(X_in, _trace=False, _tmpdir=None, _trace_cores=None):
    X = np.asarray(X_in, dtype=np.float32)
    assert X.shape == (B, L, D, N), X.shape
    nc = _get_program()
    in_maps = [{"x": s} for s in _shard(X)]
    kwargs = {}
    if _trace:
        kwargs = dict(
            trace=True,
            tmpdir=_tmpdir,
            trace_cores=_trace_cores or list(range(N_CORES)),
        )
    res = run_bass_kernel_spmd(nc, in_maps, core_ids=list(range(N_CORES)), **kwargs)
    out = _unshard([res.results[i]["y"] for i in range(N_CORES)])
    kernel.last_results = res
    return out



# revision 2
# speedup vs baseline: 1.2340x; 1.2340x over previous
"""Trainium2 Bass kernel: inclusive cumsum along L for X (4, 8192, 32, 32) f32.

Matmul-cumsum design (8 NeuronCores, SPMD), bf16 I/O:
  - Shard (batch b, L-half h): core i=(2b+h) gets slab X[b, h*4096:(h+1)*4096, :]
    viewed as (4096, 1024), converted to bf16 on the host (2 KiB DRAM rows).
    Each core computes the full cumsum of its slab; the host joins the two
    halves of a batch during unshard with one f32 broadcast add of half-0's
    last row (the sharding seam), after upcasting to f32.
  - Per core, 32 L-blocks of (128, 1024). Local block cumsum = PE matmuls
    with a constant 128x128 upper-triangular-ones bf16 matrix as stationary
    weights in 512-wide chunks (PSUM bank limit): out[m,c] = sum_{k<=m} X[k,c]
    accumulated in f32 PSUM. ACT copies PSUM -> a big SBUF staging tile (loc)
    in bf16; row 127 of each block's local cumsum is the block colsum.
  - Block offsets, per superblock of 8 blocks: one tiny SBUF->SBUF DMA
    gathers the 8 colsums into cm[9,1024] = [carry; colsum_0..7]. For each
    block, one K=9 matmul with lhsT = triA_rep[:, b*128:(b+1)*128] (the
    offset-selector column for block b replicated 128x) emits that block's
    offset row replicated across all 128 partitions, straight into f32 PSUM
    -- offsets are computed AND partition-broadcast in a single cheap matmul.
    A K=9,M=1 matmul emits the next superblock carry; DVE converts it to
    cm[0] of superblock s+1.
  - Final: DVE tensor_tensor add (in0 = loc bf16 SBUF, in1 = replicated
    offsets f32 PSUM) -> yt bf16, DMA out 2 blocks (512 KiB) at a time.
  - Measured ~90 us on 8 cores (baseline transpose+scan design: 112 us).
    Engine profile: PE ~65 us (the instruction-stream backbone; the chip
    power-throttles PE to ~1.1 ns/col), DMA ~59 us, DVE ~39 us, ACT ~36 us.
"""

import numpy as np
import ml_dtypes
from contextlib import ExitStack

import concourse.bass as bass
import concourse.tile as tile
from concourse import bacc, mybir
from concourse.bass_utils import run_bass_kernel_spmd

BF16 = ml_dtypes.bfloat16

N_CORES = 8
B, L, D, N = 4, 8192, 32, 32
C = D * N               # 1024 columns
LH = L // 2             # 4096 rows per core (L-half)
P = 128                 # partitions / L-block rows
NBLK = LH // P          # 32 L-blocks per core
SB = 8                  # blocks per superblock (offset batch)
NSUP = NBLK // SB       # 4 superblocks per core
GRP = 2                 # L-blocks per DMA (in and out)
CH = C // 512           # 512-wide matmul chunks per block

_CACHE = {}


def _build_program():
    f32 = mybir.dt.float32
    bf16 = mybir.dt.bfloat16
    nc = bacc.Bacc(
        trn_type="TRN2", debug=False, num_devices=N_CORES, num_swdge_queues=2
    )
    x = nc.dram_tensor("x", [LH, C], bf16, kind="ExternalInput").ap()
    tri = nc.dram_tensor("tri", [P, P], bf16, kind="ExternalInput").ap()
    triar = nc.dram_tensor("triar", [SB + 1, SB * P], bf16, kind="ExternalInput").ap()
    triac = nc.dram_tensor("triac", [SB + 1, 1], bf16, kind="ExternalInput").ap()
    y = nc.dram_tensor("y", [LH, C], bf16, kind="ExternalOutput").ap()

    with tile.TileContext(nc) as tc, ExitStack() as ctx:
        const_pool = ctx.enter_context(tc.tile_pool(name="const", bufs=1))
        xin_pool = ctx.enter_context(tc.tile_pool(name="xin", bufs=4))
        yout_pool = ctx.enter_context(tc.tile_pool(name="yout", bufs=4))
        cmat_pool = ctx.enter_context(tc.tile_pool(name="cmat", bufs=2))
        mmps_pool = ctx.enter_context(tc.tile_pool(name="mmps", bufs=2, space="PSUM"))
        auxps_pool = ctx.enter_context(tc.tile_pool(name="auxps", bufs=2, space="PSUM"))

        tri_sb = const_pool.tile([P, P], bf16, name="tri_sb")
        triar_sb = const_pool.tile([SB + 1, SB * P], bf16, name="triar_sb")
        triac_sb = const_pool.tile([SB + 1, 1], bf16, name="triac_sb")
        nc.sync.dma_start(out=tri_sb[:], in_=tri)
        nc.sync.dma_start(out=triar_sb[:], in_=triar)
        nc.sync.dma_start(out=triac_sb[:], in_=triac)

        loc = const_pool.tile([P, NBLK * C], bf16, name="loc")  # local cumsums

        cmats = []
        for s in range(NSUP):
            cm = cmat_pool.tile([SB + 1, C], bf16, name=f"cm{s}", tag="cm", bufs=2)
            cmats.append(cm)
            if s == 0:
                nc.gpsimd.memset(cm[0:1, :], 0.0)

        for s in range(NSUP):
            # ---- local cumsums for this superblock ----
            for g in range(SB // GRP):
                blk0 = s * SB + g * GRP
                xt = xin_pool.tile([P, GRP * C], bf16, name="xt", tag="xt", bufs=4)
                src = x[blk0 * P : (blk0 + GRP) * P, :].rearrange(
                    "(ks p) c -> p ks c", p=P
                )
                nc.sync.dma_start(out=xt[:].rearrange("p (ks c) -> p ks c", ks=GRP),
                                  in_=src)
                for ks in range(GRP):
                    b = blk0 + ks
                    ps = mmps_pool.tile([P, C], f32, name="ps", tag="ps", bufs=2)
                    for ch in range(CH):
                        nc.tensor.matmul(
                            out=ps[:, ch * 512 : (ch + 1) * 512],
                            lhsT=tri_sb[:],
                            rhs=xt[:, ks * C + ch * 512 : ks * C + (ch + 1) * 512],
                        )
                    nc.scalar.copy(loc[:, b * C : (b + 1) * C], ps[:])

            # ---- colsum gather + carry for this superblock ----
            cm = cmats[s]
            nc.gpsimd.dma_start(
                out=cm[1 : SB + 1, :],
                in_=loc[P - 1 : P, s * SB * C : (s + 1) * SB * C].rearrange(
                    "one (m c) -> one m c", m=SB
                ),
            )
            if s + 1 < NSUP:
                cp = auxps_pool.tile([1, C], f32, name="cp", tag="rp", bufs=2)
                for ch in range(CH):
                    nc.tensor.matmul(
                        out=cp[:, ch * 512 : (ch + 1) * 512],
                        lhsT=triac_sb[:],
                        rhs=cm[:, ch * 512 : (ch + 1) * 512],
                    )
                nc.vector.tensor_copy(cmats[s + 1][0:1, :], cp[:])

            # ---- replicated-offset matmuls + final adds + DMA out ----
            for g in range(SB // GRP):
                blk0 = s * SB + g * GRP
                yt = yout_pool.tile([P, GRP * C], bf16, name="yt", tag="yt", bufs=4)
                for ks in range(GRP):
                    b = blk0 + ks
                    bs = b - s * SB
                    rp = auxps_pool.tile([P, C], f32, name="rp", tag="rp", bufs=2)
                    for ch in range(CH):
                        nc.tensor.matmul(
                            out=rp[:, ch * 512 : (ch + 1) * 512],
                            lhsT=triar_sb[:, bs * P : (bs + 1) * P],
                            rhs=cm[:, ch * 512 : (ch + 1) * 512],
                        )
                    nc.vector.tensor_tensor(
                        out=yt[:, ks * C : (ks + 1) * C],
                        in0=loc[:, b * C : (b + 1) * C],
                        in1=rp[:],
                        op=mybir.AluOpType.add,
                    )
                ydst = y[blk0 * P : (blk0 + GRP) * P, :].rearrange(
                    "(ks p) c -> p ks c", p=P
                )
                out_eng = nc.gpsimd if g % 2 == 0 else nc.sync
                out_eng.dma_start(
                    out=ydst, in_=yt[:].rearrange("p (ks c) -> p ks c", ks=GRP)
                )

    nc.compile()
    return nc


def _get_program():
    if "nc" not in _CACHE:
        _CACHE["nc"] = _build_program()
    return _CACHE["nc"]


def _consts():
    tri = np.triu(np.ones((P, P), np.float32)).astype(BF16)  # tri[k,m]=1 for k<=m
    # triA[k, b]: offset selector for block b: carry (k=0) + colsum_a (k=1+a, a<b)
    tria = np.zeros((SB + 1, SB + 1), np.float32)
    tria[0, :] = 1.0
    for a in range(SB):
        tria[1 + a, a + 1 :] = 1.0
    triar = np.repeat(tria[:, :SB], P, axis=1).astype(BF16)   # [9, 8*128]
    triac = np.ones((SB + 1, 1), np.float32).astype(BF16)     # next carry selector
    return tri, triar, triac


def kernel(X_in, _trace=False, _tmpdir=None, _trace_cores=None):
    X = np.asarray(X_in, dtype=np.float32)
    assert X.shape == (B, L, D, N), X.shape
    Xv = X.reshape(B, L, C)
    tri, triar, triac = _consts()
    nc = _get_program()
    in_maps = []
    for i in range(N_CORES):
        b, h = i // 2, i % 2
        slab = np.ascontiguousarray(Xv[b, h * LH : (h + 1) * LH, :]).astype(BF16)
        in_maps.append({"x": slab, "tri": tri, "triar": triar, "triac": triac})
    kwargs = {}
    if _trace:
        kwargs = dict(
            trace=True,
            tmpdir=_tmpdir,
            trace_cores=_trace_cores or list(range(N_CORES)),
        )
    res = run_bass_kernel_spmd(nc, in_maps, core_ids=list(range(N_CORES)), **kwargs)
    out = np.empty((B, L, C), np.float32)
    for i in range(N_CORES):
        b, h = i // 2, i % 2
        out[b, h * LH : (h + 1) * LH, :] = res.results[i]["y"].astype(np.float32)
    for b in range(B):
        out[b, LH:, :] += out[b, LH - 1 : LH, :]
    kernel.last_results = res
    return out.reshape(B, L, D, N)
